# revision 16
# baseline (speedup 1.0000x reference)
import math
import numpy as np

NHEADS = 16
KVHEADS = 4
D = 128
THETA = 10000.0
B, S, E = 2, 2048, 2048
NQ = 4  # sequence quarters of 512
REPS = 512  # on-device repetitions per dispatch (amortizes launch overhead)

_CACHE = {}


def _build(stage="full", reps=REPS):
    import concourse.bacc as bacc
    from concourse import mybir
    from concourse.tile import TileContext
    from concourse import masks as cmasks
    from concourse import bass_isa

    f16 = mybir.dt.float16
    f32 = mybir.dt.float32
    EXP = mybir.ActivationFunctionType.Exp
    COPY = mybir.ActivationFunctionType.Copy
    GE = mybir.AluOpType.is_ge
    ET = mybir.EngineType

    nc = bacc.Bacc(None, target_bir_lowering=False)
    xc_d = nc.declare_dram_parameter("xc", [128, NQ * 16 * 512], f16, isOutput=False)
    wq_d = nc.declare_dram_parameter("wqc", [128, 16 * 512], f16, isOutput=False)
    wk_d = nc.declare_dram_parameter("wkc", [128, 16 * 128], f16, isOutput=False)
    wv_d = nc.declare_dram_parameter("wvc", [128, 16 * 128], f16, isOutput=False)
    wd_d = nc.declare_dram_parameter("wdc", [128, 4 * 2048], f16, isOutput=False)
    cos_d = nc.declare_dram_parameter("cosT", [128, 2048], f16, isOutput=False)
    sin_d = nc.declare_dram_parameter("sinT", [128, 2048], f16, isOutput=False)
    out_d = nc.declare_dram_parameter("out", [NQ, 128, 4 * 2048], f16, isOutput=True)

    xc_r = xc_d.rearrange("p (q e s) -> p q e s", q=NQ, e=16)
    wq_r = wq_d.rearrange("p (e c) -> p e c", e=16)
    wk_r = wk_d.rearrange("p (e c) -> p e c", e=16)
    wv_r = wv_d.rearrange("p (e c) -> p e c", e=16)
    wd_r = wd_d.rearrange("p (h e) -> p h e", h=4)

    with TileContext(nc) as tc:
        with tc.tile_pool(name="persist", bufs=1) as pp, tc.tile_pool(
            name="xpool", bufs=2
        ) as xpool, tc.tile_pool(name="qpool", bufs=6) as qpool, tc.tile_pool(
            name="epool", bufs=4
        ) as epool, tc.tile_pool(name="scr", bufs=3) as scr, tc.tile_pool(
            name="apool", bufs=10
        ) as apool, tc.tile_pool(name="opool", bufs=3) as opool, tc.tile_pool(
            name="pmm", bufs=3, space="PSUM"
        ) as pmm, tc.tile_pool(name="pu", bufs=2, space="PSUM") as pu, tc.For_i(
            0,
            reps,
            1,
            hint_engines=(ET.PE, ET.DVE, ET.Activation, ET.Pool, ET.SP),
            staggered_reset=True,
        ):
            # ---- persistent tiles ----
            wq_sb = pp.tile([128, 16, 512], f16, tag="wq")
            wk_sb = pp.tile([128, 16, 128], f16, tag="wk")
            wv_sb = pp.tile([128, 16, 128], f16, tag="wv")
            wd_sb = pp.tile([128, 4, 2048], f16, tag="wd")
            cos_sb = pp.tile([128, 2048], f16, tag="cos")
            sin_sb = pp.tile([128, 2048], f16, tag="sin")
            kT_sb = pp.tile([128, 2048], f16, tag="kT")
            v_sb = pp.tile([128, 16, 128], f16, tag="v")
            id_sb = pp.tile([128, 128], f16, tag="ident")
            msk_sb = pp.tile([128, 4, 512], f16, tag="msk")

            cmasks.make_identity(nc, id_sb[:])
            nc.gpsimd.memset(msk_sb[:], 1.0)
            nc.gpsimd.affine_select(
                out=msk_sb[:],
                in_=msk_sb[:],
                compare_op=GE,
                fill=0.0,
                base=0,
                pattern=[[-128, 4], [1, 512]],
                channel_multiplier=-1,
            )

            x_tiles = {}

            def load_x(sq, split=False):
                t = xpool.tile([128, 16, 512], f16, tag="xq", name=f"x{sq}")
                if split:
                    nc.sync.dma_start(out=t[:, 0:8, :], in_=xc_r[:, sq, 0:8, :])
                    return t
                nc.sync.dma_start(out=t[:], in_=xc_r[:, sq, :, :])
                x_tiles[sq] = t

            # startup loads: interleave x0/wq quarters so proj(0) starts earliest
            x0 = xpool.tile([128, 16, 512], f16, tag="xq", name="x0")
            x_tiles[0] = x0
            for eb0, eb1 in [(0, 2), (2, 4), (4, 8), (8, 12), (12, 16)]:
                nc.sync.dma_start(out=x0[:, eb0:eb1, :], in_=xc_r[:, 0, eb0:eb1, :])
                nc.scalar.dma_start(out=wq_sb[:, eb0:eb1, :], in_=wq_r[:, eb0:eb1, :])
                if eb0 == 2:
                    # wk/wv early: the pkv chain starts right after pq01/pq23
                    nc.scalar.dma_start(out=wk_sb[:], in_=wk_r[:])
                    nc.scalar.dma_start(out=wv_sb[:], in_=wv_r[:])
            nc.sync.dma_start(out=cos_sb[:], in_=cos_d[:])
            nc.scalar.dma_start(out=sin_sb[:], in_=sin_d[:])
            load_x(1)
            nc.scalar.dma_start(out=wd_sb[:], in_=wd_r[:])

            def rope_extract(ps_ap, nm):
                qs = scr.tile([128, 512], f16, tag="qs", bufs=5, name=f"qs{nm}")
                nc.vector.tensor_copy(qs[:], ps_ap)
                return qs

            def rope_math(qs, out_ap, sq, nm):
                cs = cos_sb[:, sq * 512 : (sq + 1) * 512]
                sn = sin_sb[:, sq * 512 : (sq + 1) * 512]
                qsh = scr.tile([128, 512], f16, tag="qsh", name=f"qsh{nm}")
                nc.vector.tensor_copy(qsh[0:64, :], qs[64:128, :])
                nc.vector.tensor_copy(qsh[64:128, :], qs[0:64, :])
                tc_ = scr.tile([128, 512], f16, tag="tc", name=f"tc{nm}")
                nc.vector.tensor_mul(tc_[:], qs[:], cs)
                ts_ = scr.tile([128, 512], f16, tag="ts", name=f"ts{nm}")
                nc.vector.tensor_mul(ts_[:], qsh[:], sn)
                nc.vector.tensor_add(out_ap, tc_[:], ts_[:])

            qt = [None] * 4

            def proj(sq):
                xh = x_tiles[sq]
                if sq + 1 < NQ and sq >= 1:
                    load_x(sq + 1)
                pq01 = pmm.tile([128, 1024], f32, tag="mm", name=f"pq01_{sq}")
                pq23 = pmm.tile([128, 1024], f32, tag="mm", name=f"pq23_{sq}")
                pkv = pmm.tile([128, 1024], f32, tag="mm", name=f"pkv_{sq}")
                # chain-major: pq01 stops first so rope(q0) overlaps later chains
                for eb in range(16):
                    st = eb == 0
                    sp = eb == 15
                    nc.tensor.matmul(
                        pq01[:, 0:512], wq_sb[:, eb, 0:128], xh[:, eb, :], start=st, stop=sp
                    )
                    nc.tensor.matmul(
                        pq01[:, 512:1024], wq_sb[:, eb, 128:256], xh[:, eb, :], start=st, stop=sp
                    )
                for eb in range(16):
                    st = eb == 0
                    sp = eb == 15
                    nc.tensor.matmul(
                        pq23[:, 0:512], wq_sb[:, eb, 256:384], xh[:, eb, :], start=st, stop=sp
                    )
                    nc.tensor.matmul(
                        pq23[:, 512:1024], wq_sb[:, eb, 384:512], xh[:, eb, :], start=st, stop=sp
                    )
                for eb in range(16):
                    st = eb == 0
                    sp = eb == 15
                    nc.tensor.matmul(
                        pkv[:, 0:512], wk_sb[:, eb, :], xh[:, eb, :], start=st, stop=sp
                    )
                    nc.tensor.matmul(
                        pkv[:, 512:1024], wv_sb[:, eb, :], xh[:, eb, :], start=st, stop=sp
                    )
                # interleave extract+math so qt[0] is ready before proj MMs end

                def do_q(j):
                    qs_j = rope_extract(
                        (pq01, pq23)[j // 2][:, (j % 2) * 512 : (j % 2) * 512 + 512],
                        f"{sq}_{j}",
                    )
                    q_t = qpool.tile([128, 512], f16, tag="qt", name=f"q{sq}_{j}")
                    rope_math(qs_j, q_t[:], sq, f"{sq}_{j}")
                    qt[j] = q_t

                def do_k():
                    vt_ = scr.tile([128, 512], f16, tag="vt", name=f"vt{sq}")
                    nc.scalar.activation(vt_[:], pkv[:, 512:1024], COPY)
                    ks = rope_extract(pkv[:, 0:512], f"{sq}_k")
                    rope_math(ks, kT_sb[:, sq * 512 : (sq + 1) * 512], sq, f"{sq}_k")
                    return vt_

                if sq == 0:
                    do_q(0)
                    vt = do_k()
                    for j in (1, 2, 3):
                        do_q(j)
                else:
                    for j in range(4):
                        do_q(j)
                    vt = do_k()

                # v transpose into [s, vd]; for sq >= 1 this is deferred into
                # the next attn()'s filler stream so a transpose waiting on
                # vt_ can't head-block the strict-FIFO PE queue.
                def v_trans():
                    tp = pmm.tile([128, 512], f16, tag="mm", name=f"tp{sq}")
                    for t4 in range(4):
                        nc.tensor.transpose(
                            tp[:, t4 * 128 : (t4 + 1) * 128],
                            vt[:, t4 * 128 : (t4 + 1) * 128],
                            id_sb[:],
                        )
                    nc.scalar.activation(v_sb[:, 4 * sq : 4 * sq + 4, :], tp[:], COPY)

                if sq == 0:
                    v_trans()
                    return None
                return v_trans

            a_tiles = {}

            def attn(sq, fillers=()):
                fillers = list(fillers)
                nk = 4 * sq + 4
                pairs = [(h, kp) for h in range(4) for kp in range(0, nk, 2)]
                fill_every = max(1, len(pairs) // 8)

                def make_pair(h, kp):
                    if kp == 4 * sq + 2:
                        # second diagonal pair: queries < 256 are fully masked
                        # for these k blocks — compute only the upper 256 cols.
                        sp_ = pmm.tile(
                            [128, 512], f32, tag="mm", name=f"s{sq}_{h}_{kp}"
                        )
                        nc.tensor.matmul(
                            sp_[:, 0:256],
                            kT_sb[:, kp * 128 : (kp + 1) * 128],
                            qt[h][:, 256:512],
                            start=True,
                            stop=True,
                        )
                        nc.tensor.matmul(
                            sp_[:, 256:512],
                            kT_sb[:, (kp + 1) * 128 : (kp + 2) * 128],
                            qt[h][:, 256:512],
                            start=True,
                            stop=True,
                        )
                        ep = epool.tile(
                            [128, 512], f16, tag="ep", name=f"e{sq}_{h}_{kp}"
                        )
                        nc.scalar.activation(ep[:], sp_[:], EXP)
                        em = epool.tile(
                            [128, 512], f16, tag="em", bufs=3, name=f"em{sq}_{h}_{kp}"
                        )
                        nc.vector.tensor_mul(
                            em.rearrange("p (b q) -> p b q", b=2)[:],
                            ep.rearrange("p (b q) -> p b q", b=2)[:],
                            msk_sb[:, 2:4, 256:512],
                        )
                        return (em, True)
                    sp_ = pmm.tile([128, 1024], f32, tag="mm", name=f"s{sq}_{h}_{kp}")
                    nc.tensor.matmul(
                        sp_[:, 0:512],
                        kT_sb[:, kp * 128 : (kp + 1) * 128],
                        qt[h][:],
                        start=True,
                        stop=True,
                    )
                    nc.tensor.matmul(
                        sp_[:, 512:1024],
                        kT_sb[:, (kp + 1) * 128 : (kp + 2) * 128],
                        qt[h][:],
                        start=True,
                        stop=True,
                    )
                    ep = epool.tile([128, 1024], f16, tag="ep", name=f"e{sq}_{h}_{kp}")
                    nc.scalar.activation(ep[:], sp_[:], EXP)
                    if kp >= 4 * sq:
                        m0 = kp - 4 * sq
                        em = epool.tile(
                            [128, 1024], f16, tag="em", bufs=3, name=f"em{sq}_{h}_{kp}"
                        )
                        nc.vector.tensor_mul(
                            em.rearrange("p (b q) -> p b q", b=2)[:],
                            ep.rearrange("p (b q) -> p b q", b=2)[:],
                            msk_sb[:, m0 : m0 + 2, :],
                        )
                        return (em, False)
                    return (ep, False)

                u_ps = None
                esum = None
                depth = 2 if fillers else 3
                e_pipe = [make_pair(*p) for p in pairs[:depth]]
                if fillers:
                    fillers.pop(0)()
                for i, (h, kp) in enumerate(pairs):
                    e_prev, half = e_pipe.pop(0)
                    if kp == 0:
                        u_ps = pu.tile([128, 512], f32, tag="u", name=f"u{sq}_{h}")
                        esum = None
                    if half:
                        nc.tensor.matmul(
                            u_ps[:, 256:512],
                            v_sb[:, kp, :],
                            e_prev[:, 0:256],
                            start=False,
                            stop=False,
                        )
                        nc.tensor.matmul(
                            u_ps[:, 256:512],
                            v_sb[:, kp + 1, :],
                            e_prev[:, 256:512],
                            start=False,
                            stop=(kp + 2 == nk),
                        )
                    else:
                        nc.tensor.matmul(
                            u_ps[:],
                            v_sb[:, kp, :],
                            e_prev[:, 0:512],
                            start=(kp == 0),
                            stop=False,
                        )
                        nc.tensor.matmul(
                            u_ps[:],
                            v_sb[:, kp + 1, :],
                            e_prev[:, 512:1024],
                            start=False,
                            stop=(kp + 2 == nk),
                        )
                    if esum is None:
                        esum = scr.tile(
                            [128, 512], f16, tag="esum", bufs=3, name=f"es{sq}_{h}_{kp}"
                        )
                        nc.vector.tensor_add(
                            esum[:], e_prev[:, 0:512], e_prev[:, 512:1024]
                        )
                    elif half:
                        psum_t = scr.tile(
                            [128, 256], f16, tag="psm", name=f"pr{sq}_{h}_{kp}"
                        )
                        nc.vector.tensor_add(
                            psum_t[:], e_prev[:, 0:256], e_prev[:, 256:512]
                        )
                        esum_new = scr.tile(
                            [128, 512], f16, tag="esum", bufs=3, name=f"es{sq}_{h}_{kp}"
                        )
                        nc.vector.tensor_copy(esum_new[:, 0:256], esum[:, 0:256])
                        nc.vector.tensor_add(
                            esum_new[:, 256:512], esum[:, 256:512], psum_t[:]
                        )
                        esum = esum_new
                    else:
                        psum_t = scr.tile(
                            [128, 512], f16, tag="psm", name=f"pr{sq}_{h}_{kp}"
                        )
                        nc.vector.tensor_add(
                            psum_t[:], e_prev[:, 0:512], e_prev[:, 512:1024]
                        )
                        esum_new = scr.tile(
                            [128, 512], f16, tag="esum", bufs=3, name=f"es{sq}_{h}_{kp}"
                        )
                        nc.vector.tensor_add(esum_new[:], esum[:], psum_t[:])
                        esum = esum_new
                    if kp + 2 == nk:
                        # softmax denominator: partition-reduce esum on the
                        # (otherwise idle) GpSimd engine, then approx recip.
                        nbc = scr.tile([128, 512], f32, tag="nbc", name=f"nb{sq}_{h}")
                        nc.gpsimd.partition_all_reduce(
                            nbc[:], esum[:], channels=128, reduce_op=bass_isa.ReduceOp.add
                        )
                        rb = scr.tile([128, 512], f32, tag="rb", name=f"rb{sq}_{h}")
                        nc.vector.reciprocal_approx_fast(out=rb[:], in_=nbc[:])
                        a_t = apool.tile([128, 512], f16, tag="a", name=f"a{sq}_{h}")
                        nc.vector.tensor_mul(a_t[:], u_ps[:], rb[:])
                        a_tiles[(sq, h)] = a_t
                    if fillers and (i + 1) % fill_every == 0:
                        fillers.pop(0)()
                    if i + depth < len(pairs):
                        e_pipe.append(make_pair(*pairs[i + depth]))
                for f in fillers:
                    f()

            osb_cur = {}

            def dense_group(sq, sc, nbp):
                def emit():
                    if nbp == 0:
                        osb_cur[sq] = opool.tile(
                            [128, 2048], f16, tag="o", name=f"osb{sq}_{sc}"
                        )
                    osb = osb_cur[sq]
                    d_ps = pmm.tile(
                        [128, 1024], f32, tag="mm", name=f"d{sq}_{sc}_{nbp}"
                    )
                    for h in range(4):
                        st = h == 0
                        spp = h == 3
                        nc.tensor.matmul(
                            d_ps[:, 0:512],
                            a_tiles[(sq, h)][:, sc * 128 : (sc + 1) * 128],
                            wd_sb[:, h, nbp * 1024 : nbp * 1024 + 512],
                            start=st,
                            stop=spp,
                        )
                        nc.tensor.matmul(
                            d_ps[:, 512:1024],
                            a_tiles[(sq, h)][:, sc * 128 : (sc + 1) * 128],
                            wd_sb[:, h, nbp * 1024 + 512 : nbp * 1024 + 1024],
                            start=st,
                            stop=spp,
                        )
                    # alternate PSUM->SBUF evacuation between ACT and DVE so
                    # d_ps slots release without queuing behind exps
                    if (sc + nbp) % 2 == 0:
                        nc.scalar.activation(
                            osb[:, nbp * 1024 : (nbp + 1) * 1024], d_ps[:], COPY
                        )
                    else:
                        nc.vector.tensor_copy(
                            osb[:, nbp * 1024 : (nbp + 1) * 1024], d_ps[:]
                        )
                    nc.sync.dma_start(
                        out=out_d[
                            sq, :, sc * 2048 + nbp * 1024 : sc * 2048 + (nbp + 1) * 1024
                        ],
                        in_=osb[:, nbp * 1024 : (nbp + 1) * 1024],
                    )
                return emit

            def dense_groups(sq):
                return [dense_group(sq, sc, nbp) for sc in range(4) for nbp in range(2)]

            def dense(sq):
                for f in dense_groups(sq):
                    f()

            if stage != "dma":
                pend = proj(0)
                for sq in range(NQ):
                    if stage in ("attn", "full"):
                        fill = [pend] if pend else []
                        if stage == "full" and sq >= 1:
                            fill += dense_groups(sq - 1)
                        attn(sq, fill)
                    if sq + 1 < NQ:
                        pend = proj(sq + 1)
                    if sq < NQ - 1:
                        tc.stage_boundary()
                if stage == "full":
                    dense(NQ - 1)
            else:
                load_x(2)
                load_x(3)
    nc.compile()
    return nc


def _host_inputs(x, w_qkv, w_dense):
    scale = 1.0 / math.sqrt(D)
    pos = np.arange(S, dtype=np.float64)
    invf = 1.0 / (THETA ** (np.arange(0, D, 2, dtype=np.float64) / D))
    ang = pos[:, None] * invf[None, :]  # [S, 64]
    cos_h = np.cos(ang)
    sin_h = np.sin(ang)
    cosT = np.concatenate([cos_h, cos_h], axis=1).T  # [128, S]
    sinT = np.concatenate([sin_h, sin_h], axis=1).T
    sinT[0:64, :] *= -1.0
    cosT16 = np.ascontiguousarray(cosT.astype(np.float16))
    sinT16 = np.ascontiguousarray(sinT.astype(np.float16))

    x16 = x.astype(np.float16)
    wqkv16 = w_qkv.astype(np.float16)
    wd16 = w_dense.astype(np.float16)

    in_maps = []
    xc_cache = {}
    for c in range(8):
        b, g = divmod(c, 4)
        if b not in xc_cache:
            xb = x16[b]  # [S, E]
            xc = np.ascontiguousarray(
                xb.reshape(NQ, 512, 16, 128).transpose(3, 0, 2, 1)
            ).reshape(128, -1)
            xc_cache[b] = xc
        wq = (
            (w_qkv[:, 512 * g : 512 * (g + 1)] * np.float32(scale))
            .astype(np.float16)
            .reshape(16, 128, 512)
            .transpose(1, 0, 2)
        )
        wk = (
            wqkv16[:, 2048 + 128 * g : 2048 + 128 * (g + 1)]
            .reshape(16, 128, 128)
            .transpose(1, 0, 2)
        )
        wv = (
            wqkv16[:, 2560 + 128 * g : 2560 + 128 * (g + 1)]
            .reshape(16, 128, 128)
            .transpose(1, 0, 2)
        )
        wd = wd16[512 * g : 512 * (g + 1)].reshape(4, 128, 2048).transpose(1, 0, 2)
        in_maps.append(
            {
                "xc": xc_cache[b],
                "wqc": np.ascontiguousarray(wq).reshape(128, -1),
                "wkc": np.ascontiguousarray(wk).reshape(128, -1),
                "wvc": np.ascontiguousarray(wv).reshape(128, -1),
                "wdc": np.ascontiguousarray(wd).reshape(128, -1),
                "cosT": cosT16,
                "sinT": sinT16,
            }
        )
    return in_maps


def _assemble(outs):
    full = np.empty((B, S, E), np.float32)
    for b in range(B):
        acc = (
            outs[4 * b].astype(np.float32)
            + outs[4 * b + 1].astype(np.float32)
            + outs[4 * b + 2].astype(np.float32)
            + outs[4 * b + 3].astype(np.float32)
        )
        full[b] = (
            acc.reshape(NQ, 128, 4, 2048).transpose(0, 2, 1, 3).reshape(S, E)
        )
    return full


def kernel(x, w_qkv, w_dense):
    import concourse.bass_utils as bass_utils

    if "nc" not in _CACHE:
        _CACHE["nc"] = _build()
    nc = _CACHE["nc"]
    in_maps = _host_inputs(x, w_qkv, w_dense)
    res = bass_utils.run_bass_kernel_spmd(nc, in_maps, list(range(8)), trace=False)
    outs = [np.asarray(res.results[c]["out"]) for c in range(8)]
    return _assemble(outs)
